# revision 1
# baseline (speedup 1.0000x reference)
"""Bahdanau attention Trainium2 kernel.

Problem: B=8, T=256, S=256, H=512 (fp32 I/O).
  Ws_q = q @ W_s.T ; Wh_e = e @ W_h.T
  energies[b,t,s] = v . tanh(Ws_q[b,t,:] + Wh_e[b,s,:])   (masked s >= len_b)
  attn = softmax_s(energies); ctx = attn @ e
  out = tanh(concat([ctx, q]) @ W_out.T)

Sharding: sequence-parallel over T — core c handles t in [c*32, (c+1)*32)
for ALL batches, balancing src_lengths sparsity across cores.

Per-core dataflow (bf16 compute, fp32 accumulation):
  PE   : Ws_q^T [o,t] and Wh_e^T [o,s] projections (o on partitions)
  DVE  : X[o, t, s] = es[o,s] + qs[o,t] (per-t tensor_scalar into
         per-(oc, t-half) half-tiles so each tanh overlaps the next
         half's adds; one shared tile with sliced ops falsely serializes)
  ACT  : tanh(X) in place, one instruction per (b, oc, half) [F = 16*L]
  PE   : energies[t,s] = sum_o v_o X[o,t,s] — M=1 matmuls col-tiled 4-wide
  DMA  : gather PSUM rows {0,32,64,96} -> energies [32t, s]
  DVE/ACT: masked softmax (exp over the first len_b cols + zeroed weight
         tail, with accum_out for the row sums)
  DMA  : xbar-transpose of weights [32,s] -> [s,32]
  PE   : ctx^T[h,t] = enc^T @ w^T ; out[t,o] = tanh(comb^T.T @ W_out^T)

The kernel is DVE-bound: the 1024 per-t adds cost ~200ns each in
context (~205us), with ACT at ~195us and PE at ~135us. Measured HW
instruction rates (big-loop slope microbenches): DVE tensor_scalar
~165-260ns for F=128-512 (mostly fixed cost); ACT ~145ns + ~1.0ns/
free-elem; GPSIMD ~230ns + 1.6ns/elem.

Rebalancing attempts that measurably HURT on HW (interleaved A/B, all
reverted — engine streams are in-order, and cross-engine sync costs
dominate fine-grained offload):
  - GPSIMD broadcast adds for any slice of X (whole oc, halves, or 2-4
    t's per half): +15-50us. GPSIMD sem waits/completion are ~us-scale.
  - Fused bias-tanh on ACT for k of 16 t's (tanh(es + qs_t), F=L):
    monotonically worse with k (~+300ns/instr in-context, not ~145).
  - es/vscr PSUM evacs on ACT (count-neutral): +13/+36us — the copies
    queue behind ~6us tanh instructions and starve DVE/exp.
  - GPSIMD cannot read PSUM at all (BIR verifier rejects).

HW notes: PSUM accumulation groups must not interleave within a
(partition, bank) zero-region; DMA cannot read PSUM; single-DMA
free-dim->partition scatter silently misplaces data; energ-style
[4, 256] partition-gather DMA ~500ns, [32,128] xbar transpose ~1.3us.
"""

import functools

import ml_dtypes
import numpy as np

B, T, S, H = 8, 256, 256, 512
NCORES = 8
TC = T // NCORES  # 32 target positions per core
KC = H // 128     # 4 contraction chunks
OC = H // 128     # 4 output-feature chunks

_BF16 = ml_dtypes.bfloat16


def _ceil4(x: int) -> int:
    return max(4, (x + 3) // 4 * 4)


@functools.lru_cache(maxsize=8)
def _build(lens: tuple, loop_n: int | None = None, stages: int = 3):
    """Build + compile the per-core Bass program with per-batch s-extents
    baked in. Same program runs on all 8 cores (inputs differ)."""
    import concourse.mybir as mybir
    import concourse.tile as tile
    from concourse import bacc

    f32 = mybir.dt.float32
    bf16 = mybir.dt.bfloat16
    AF = mybir.ActivationFunctionType

    Ls = [_ceil4(l) for l in lens]

    nc = bacc.Bacc("TRN2", target_bir_lowering=False, debug=False)

    # All inputs are host-pre-arranged into SBUF layout [128, free].
    qt_d = nc.dram_tensor("qt", [128, KC, B, TC], bf16, kind="ExternalInput")
    encT_d = nc.dram_tensor("encT", [B, 128, KC, S], bf16, kind="ExternalInput")
    enc_d = nc.dram_tensor("enc", [B, 128, S // 128, H], bf16, kind="ExternalInput")
    wst_d = nc.dram_tensor("wst", [128, KC, H], bf16, kind="ExternalInput")
    wht_d = nc.dram_tensor("wht", [128, KC, H], bf16, kind="ExternalInput")
    v_d = nc.dram_tensor("v", [128, KC], bf16, kind="ExternalInput")
    wot_d = nc.dram_tensor("wot", [128, 2 * KC, H], bf16, kind="ExternalInput")
    out_d = nc.dram_tensor("out", [B, TC, H], f32, kind="ExternalOutput")

    import contextlib

    with tile.TileContext(nc) as tc:
        loop_cm = (
            tc.For_i(
                0, loop_n, 1,
                hint_engines=(
                    mybir.EngineType.PE, mybir.EngineType.DVE,
                    mybir.EngineType.Activation, mybir.EngineType.SP,
                    mybir.EngineType.Pool,
                ),
            )
            if loop_n is not None
            else contextlib.nullcontext()
        )
        with (
            tc.tile_pool(name="const", bufs=1) as constp,
            tc.tile_pool(name="enc", bufs=3) as encp,
            tc.tile_pool(name="es", bufs=2) as esp,
            tc.tile_pool(name="x", bufs=2) as xp,
            tc.tile_pool(name="sm", bufs=3) as smp,
            tc.tile_pool(name="outs", bufs=3) as outp,
            tc.tile_pool(name="psA", bufs=3, space="PSUM") as psA,
            tc.tile_pool(name="psV", bufs=2, space="PSUM") as psV,
            tc.tile_pool(name="psC", bufs=1, space="PSUM") as psC,
            tc.tile_pool(name="psO", bufs=1, space="PSUM") as psO,
            loop_cm,
        ):
            # ---- persistent weights/activations ----
            # DMA order matters for pipeline fill: projQ deps (qt, wst) and
            # projE deps (wht) first; v/wot are not needed until the first
            # tail.
            # Two HWDGE queues in parallel: projQ deps (qt, wst) on the SP
            # queue; projE dep (wht) + late consts (v, wot) on the ACT
            # queue, so the first Wh_e projection isn't serialized behind
            # the full weight load.
            qt_sb = constp.tile([128, KC, B, TC], bf16)
            nc.sync.dma_start(qt_sb[:], qt_d[:])
            wst = constp.tile([128, KC, H], bf16)
            nc.sync.dma_start(wst[:], wst_d[:])
            wht = constp.tile([128, KC, H], bf16)
            nc.scalar.dma_start(wht[:], wht_d[:])
            v_sb = constp.tile([128, KC], bf16)
            nc.scalar.dma_start(v_sb[:], v_d[:])
            wot = constp.tile([128, 2 * KC, H], bf16)
            nc.scalar.dma_start(wot[:], wot_d[:])

            # ---- Ws_q^T for all (b, t): qs[o-part, oc, b, t] ----
            # f32 copy feeds DVE tensor_scalar (scalar must be f32);
            # bf16 copy feeds Pool's broadcast tensor_tensor.
            qs_sb = constp.tile([128, OC, B, TC], f32)
            for oc in range(OC):
                ps = psA.tile([128, B * TC], f32, tag="proj")
                for kc in range(KC):
                    nc.tensor.matmul(
                        ps[:],
                        wst[:, kc, oc * 128 : (oc + 1) * 128],
                        qt_sb[:, kc, :, :],
                        start=(kc == 0),
                        stop=(kc == KC - 1),
                    )
                nc.scalar.copy(
                    qs_sb[:, oc, :, :], ps.rearrange("p (b t) -> p b t", b=B)
                )

            # Software-pipelined emission: engines execute their streams in
            # order, so the tail of batch b (vdot/softmax/ctx/out — gated on
            # long dependency chains) is emitted AFTER the head of batch b+1
            # (proj/adds/tanh). This keeps DVE/ACT streaming without stalls.
            state = {}

            def head(b):
                L = Ls[b]
                # load encoder (both layouts), full-S tiles for clean DMA
                encT_b = encp.tile([128, KC, S], bf16, tag="encT")
                nc.sync.dma_start(encT_b[:], encT_d[b])
                # second HWDGE engine (ACT) -> disjoint queue set; issue cost
                # in the ACT stream is negligible and it has no dependencies
                enc_b = encp.tile([128, S // 128, H], bf16, tag="enc")
                nc.scalar.dma_start(enc_b[:], enc_d[b])

                # Wh_e^T: es[o-part, s] per oc
                es = []
                for oc in range(OC):
                    ps = psA.tile([128, L], f32, tag="proj")
                    for kc in range(KC):
                        nc.tensor.matmul(
                            ps[:],
                            wht[:, kc, oc * 128 : (oc + 1) * 128],
                            encT_b[:, kc, :L],
                            start=(kc == 0),
                            stop=(kc == KC - 1),
                        )
                    e = esp.tile([128, L], bf16, tag=f"es{oc}")
                    nc.vector.tensor_copy(e[:], ps[:])
                    es.append(e)

                # X[o, t, s] = es[o, s] + qs[o, t]; tanh in place.
                # Two half-tiles per (oc): Tile tracks deps coarsely per
                # tile, so separate tiles let the tanh of half 0 overlap
                # the adds of half 1 without false serialization.
                HT = TC // 2
                X = []
                for oc in range(OC):
                    halves = []
                    for h2 in range(2):
                        x = xp.tile([128, HT, L], bf16, tag=f"x{oc}h{h2}",
                                    name=f"x{oc}h{h2}")
                        for tl in range(HT):
                            t = h2 * HT + tl
                            nc.vector.tensor_scalar_add(
                                x[:, tl, :], es[oc][:], qs_sb[:, oc, b, t : t + 1]
                            )
                        if stages != 4:
                            nc.scalar.activation(x[:], x[:], AF.Tanh)
                        halves.append(x)
                    X.append(halves)
                state[b] = (X, enc_b)

            def tail(b):
                L = Ls[b]
                ln = min(int(lens[b]), S)
                SC = (L + 127) // 128
                L128 = SC * 128
                X, enc_b = state.pop(b)
                if stages == 1:
                    ob = outp.tile([32, 16], f32, tag="ob1")
                    nc.vector.tensor_copy(ob[:], X[0][0][:32, 0, :16])
                    nc.sync.dma_start(out_d[b][:, :16], ob[:])
                    return

                # energies[t, s] = sum_o v_o X[o, t, s]: M=1 matmuls, 16 t's
                # per PSUM tile (4 col groups x 4 bank slots), wide DVE evac,
                # partition->partition DMA gather. Note: accumulation groups
                # sharing a (partition, bank) zero-region must not interleave
                # (start=True marks the whole 2KB bank-row pending-zero);
                # col groups (distinct partitions) may interleave freely.
                energ = smp.tile([32, L], f32, tag="energ")
                for h in range(TC // 8):
                    psq = psV.tile([128, 2, 512 // 2], f32, tag="vdot")
                    for n in range(2):
                        for oc in range(OC):
                            for j in range(4):
                                t = h * 8 + 4 * n + j
                                nc.tensor.matmul(
                                    psq[32 * j : 32 * j + 1, n, :L],
                                    v_sb[:, oc : oc + 1],
                                    X[oc][t // 16][:, t % 16, :],
                                    start=(oc == 0),
                                    stop=(oc == OC - 1),
                                    tile_position=(0, 32 * j),
                                )
                    vscr = smp.tile([128, 2, L], f32, tag="vscr")
                    nc.vector.tensor_copy(vscr[:], psq[:, :, :L])
                    vsr = vscr.rearrange("(g r) n f -> g r n f", r=32)
                    for n in range(2):
                        nc.sync.dma_start(
                            energ[h * 8 + 4 * n : h * 8 + 4 * n + 4, :],
                            vsr[:, 0, n, :],
                        )

                if stages == 2:
                    ob = outp.tile([32, 16], f32, tag="ob1")
                    nc.vector.tensor_copy(ob[:], energ[:, :16])
                    nc.sync.dma_start(out_d[b][:, :16], ob[:])
                    return
                # softmax over s (energies bounded by sum|v| ~ 20: raw exp
                # is safe in fp32/bf16 — skip max-subtract). The s >= len_b
                # mask is applied by exp-ing only the first ln columns and
                # zeroing the weight tail, rather than a -inf energy fill.
                w_sb = smp.tile([32, L128], bf16, tag="w")
                if L128 > ln:
                    nc.vector.memset(w_sb[:, ln:], 0.0)
                sm = smp.tile([32, 1], f32, tag="sm")
                nc.scalar.activation(
                    w_sb[:, :ln], energ[:, :ln], AF.Exp, accum_out=sm[:]
                )
                rs = smp.tile([32, 1], f32, tag="rs")
                nc.vector.reciprocal(rs[:], sm[:])
                nc.vector.tensor_scalar_mul(w_sb[:, :ln], w_sb[:, :ln], rs[:])

                # w^T via DMA xbar transpose: [32, L128] -> [L128, 32]
                wT = smp.tile([128, SC, TC], bf16, tag="wT")
                for sc in range(SC):
                    nc.sync.dma_start_transpose(
                        wT[:, sc, :], w_sb[:, sc * 128 : (sc + 1) * 128]
                    )

                # ctx^T[h, t] = sum_s enc[s, h] * w[t, s]; all 4 oc slots in
                # one PSUM bank-row (groups are sequential per slot — legal)
                psc = psC.tile([128, OC, TC], f32, tag="ctx")
                for oc in range(OC):
                    for sc in range(SC):
                        nc.tensor.matmul(
                            psc[:, oc, :],
                            enc_b[:, sc, oc * 128 : (oc + 1) * 128],
                            wT[:, sc, :],
                            start=(sc == 0),
                            stop=(sc == SC - 1),
                        )
                ctxT = outp.tile([128, OC, TC], bf16, tag="ctxT")
                nc.vector.tensor_copy(ctxT[:], psc[:])

                # out[t, o] = tanh(sum_k comb^T[k, t] * W_out[o, k])
                pso = psO.tile([32, H], f32, tag="outp")
                for kc in range(2 * KC):
                    lhsT = (
                        ctxT[:, kc, :]
                        if kc < OC
                        else qt_sb[:, kc - OC, b, :]
                    )
                    nc.tensor.matmul(
                        pso[:],
                        lhsT,
                        wot[:, kc, :],
                        start=(kc == 0),
                        stop=(kc == 2 * KC - 1),
                    )
                ob = outp.tile([32, H], f32, tag="ob")
                nc.scalar.activation(ob[:], pso[:], AF.Tanh)
                nc.sync.dma_start(out_d[b], ob[:])

            # Descending-L order: the pipeline tail drain (last batch's
            # tail with no head to overlap) is paid on the smallest batch.
            order = sorted(range(B), key=lambda b: -Ls[b])
            for i, b in enumerate(order):
                head(b)
                if i > 0:
                    tail(order[i - 1])
            tail(order[-1])

    nc.compile()
    return nc


def _prep_inputs(query, encoder_outputs, src_lengths, W_s, W_h, v, W_out):
    """Host-side: cast to bf16 and pre-arrange into SBUF layouts."""
    q = np.asarray(query, np.float32)
    e = np.asarray(encoder_outputs, np.float32)

    # [128, KC, B, TC] per core: qt[p, kc, b, t] = q[b, c*TC+t, kc*128+p]
    # build once for full T then slice per core.
    qt_full = np.transpose(
        q.reshape(B, T, KC, 128), (2, 3, 0, 1)
    )  # [KC, 128, B, T]
    qt_full = np.ascontiguousarray(np.swapaxes(qt_full, 0, 1)).astype(_BF16)
    # -> [128, KC, B, T]

    # encT[b, p, kc, s] = e[b, s, kc*128+p]
    encT = np.ascontiguousarray(
        np.transpose(e.reshape(B, S, KC, 128), (0, 3, 2, 1))
    ).astype(_BF16)
    # enc[b, p, sc, h] = e[b, sc*128+p, h]
    enc = np.ascontiguousarray(
        np.transpose(e.reshape(B, S // 128, 128, H), (0, 2, 1, 3))
    ).astype(_BF16)

    # wst[p, kc, o] = W_s[o, kc*128+p]
    wst = np.ascontiguousarray(
        np.transpose(np.asarray(W_s, np.float32).reshape(H, KC, 128), (2, 1, 0))
    ).astype(_BF16)
    wht = np.ascontiguousarray(
        np.transpose(np.asarray(W_h, np.float32).reshape(H, KC, 128), (2, 1, 0))
    ).astype(_BF16)
    # v[p, kc] = v[kc*128+p]
    v_pre = np.ascontiguousarray(
        np.asarray(v, np.float32).reshape(KC, 128).T
    ).astype(_BF16)
    # wot[p, kc, o] = W_out[o, kc*128+p]   (k = 2H contraction)
    wot = np.ascontiguousarray(
        np.transpose(np.asarray(W_out, np.float32).reshape(H, 2 * KC, 128), (2, 1, 0))
    ).astype(_BF16)

    lens = tuple(int(x) for x in np.asarray(src_lengths).reshape(-1))
    return qt_full, encT, enc, wst, wht, v_pre, wot, lens


def kernel(query, encoder_outputs, src_lengths, W_s, W_h, v, W_out):
    from concourse import bass_utils

    qt_full, encT, enc, wst, wht, v_pre, wot, lens = _prep_inputs(
        query, encoder_outputs, src_lengths, W_s, W_h, v, W_out
    )
    nc = _build(lens)

    in_maps = []
    for c in range(NCORES):
        qt_c = np.ascontiguousarray(
            qt_full[:, :, :, c * TC : (c + 1) * TC]
        )
        in_maps.append(
            {
                "qt": qt_c,
                "encT": encT,
                "enc": enc,
                "wst": wst,
                "wht": wht,
                "v": v_pre,
                "wot": wot,
            }
        )

    res = bass_utils.run_bass_kernel_spmd(nc, in_maps, core_ids=list(range(NCORES)))

    out = np.empty((B, T, H), np.float32)
    for c in range(NCORES):
        out[:, c * TC : (c + 1) * TC, :] = res.results[c]["out"]
    return out



# revision 3
# speedup vs baseline: 1.2034x; 1.2034x over previous
"""Bahdanau attention Trainium2 kernel.

Problem: B=8, T=256, S=256, H=512 (fp32 I/O).
  Ws_q = q @ W_s.T ; Wh_e = e @ W_h.T
  energies[b,t,s] = v . tanh(Ws_q[b,t,:] + Wh_e[b,s,:])   (masked s >= len_b)
  attn = softmax_s(energies); ctx = attn @ e
  out = tanh(concat([ctx, q]) @ W_out.T)

Sharding: sequence-parallel over T — core c handles t in [c*32, (c+1)*32)
for ALL batches, balancing src_lengths sparsity across cores.

Per-core dataflow (bf16 compute, fp32 accumulation):
  PE   : Ws_q^T [o,t] and Wh_e^T [o,s] projections (o on partitions)
  DVE  : X[o, t, s] = es[o,s] + qs[o,t] via BROADCAST tensor_tensor:
         es with stride-0 over t, qs pre-duplicated into pairs
         (qs2[o,t,2]) so the qs operand's innermost AP dim is [1,2] —
         this keeps every operand 2-byte/step-1 and the DVE in 2x_1P
         mode (~0.52 ns/free-elem vs 0.83 for per-t tensor_scalar).
         Chunked G t's per instruction (F = G*L in [1024, 2048]).
  ACT  : tanh(X) in place, F = 16L per instr (bf16 activation runs at
         ~2 elem/cycle; F~4096 is the sweet spot at ~0.53 ns/elem)
  PE   : energies[t,s] = sum_o v_o X[o,t,s] — M=1 matmuls col-tiled 4-wide
  DMA  : gather PSUM rows {0,32,64,96} -> energies [32t, s]
  DVE/ACT: masked softmax (exp over the first len_b cols + zeroed weight
         tail, with accum_out for the row sums)
  DMA  : xbar-transpose of weights [32,s] -> [s,32]
  PE   : ctx^T[h,t] = enc^T @ w^T ; out[t,o] = tanh(comb^T.T @ W_out^T)

Emission interleaves tail(b-1) segments between head(b)'s per-oc adds so
each engine's in-order queue reaches tail work early (vdot/vscr after
oc1, softmax after oc2, ctx/out after oc3) instead of queueing the whole
tail behind 30us of adds.

HW notes: PSUM accumulation groups must not interleave within a
(partition, bank) zero-region; DMA cannot read PSUM; single-DMA
free-dim->partition scatter silently misplaces data; energ-style
[4, 256] partition-gather DMA ~500ns, [32,128] xbar transpose ~1.3us.
"""

import functools

import ml_dtypes
import numpy as np

B, T, S, H = 8, 256, 256, 512
NCORES = 8
TC = T // NCORES  # 32 target positions per core
KC = H // 128     # 4 contraction chunks
OC = H // 128     # 4 output-feature chunks

_BF16 = ml_dtypes.bfloat16


def _ceil4(x: int) -> int:
    return max(4, (x + 3) // 4 * 4)


@functools.lru_cache(maxsize=8)
def _build(lens: tuple, loop_n: int | None = None, stages: int = 3):
    """Build + compile the per-core Bass program with per-batch s-extents
    baked in. Same program runs on all 8 cores (inputs differ)."""
    import concourse.mybir as mybir
    import concourse.tile as tile
    from concourse import bacc

    f32 = mybir.dt.float32
    bf16 = mybir.dt.bfloat16
    AF = mybir.ActivationFunctionType

    Ls = [_ceil4(l) for l in lens]

    nc = bacc.Bacc("TRN2", target_bir_lowering=False, debug=False)

    # All inputs are host-pre-arranged into SBUF layout [128, free].
    qt_d = nc.dram_tensor("qt", [128, KC, B, TC], bf16, kind="ExternalInput")
    encT_d = nc.dram_tensor("encT", [B, 128, KC, S], bf16, kind="ExternalInput")
    enc_d = nc.dram_tensor("enc", [B, 128, S // 128, H], bf16, kind="ExternalInput")
    wst_d = nc.dram_tensor("wst", [128, KC, H], bf16, kind="ExternalInput")
    wht_d = nc.dram_tensor("wht", [128, KC, H], bf16, kind="ExternalInput")
    v_d = nc.dram_tensor("v", [128, KC], bf16, kind="ExternalInput")
    wot_d = nc.dram_tensor("wot", [128, 2 * KC, H], bf16, kind="ExternalInput")
    out_d = nc.dram_tensor("out", [B, TC, H], f32, kind="ExternalOutput")

    import contextlib

    with tile.TileContext(nc) as tc:
        loop_cm = (
            tc.For_i(
                0, loop_n, 1,
                hint_engines=(
                    mybir.EngineType.PE, mybir.EngineType.DVE,
                    mybir.EngineType.Activation, mybir.EngineType.SP,
                    mybir.EngineType.Pool,
                ),
            )
            if loop_n is not None
            else contextlib.nullcontext()
        )
        with (
            tc.tile_pool(name="const", bufs=1) as constp,
            tc.tile_pool(name="enc", bufs=3) as encp,
            tc.tile_pool(name="es", bufs=2) as esp,
            tc.tile_pool(name="x", bufs=2) as xp,
            tc.tile_pool(name="sm", bufs=3) as smp,
            tc.tile_pool(name="outs", bufs=3) as outp,
            tc.tile_pool(name="psA", bufs=3, space="PSUM") as psA,
            tc.tile_pool(name="psV", bufs=2, space="PSUM") as psV,
            tc.tile_pool(name="psC", bufs=1, space="PSUM") as psC,
            tc.tile_pool(name="psO", bufs=1, space="PSUM") as psO,
            loop_cm,
        ):
            # ---- persistent weights/activations ----
            # Two HWDGE queues in parallel: projQ deps (qt, wst) on the SP
            # queue; projE dep (wht) + late consts (v, wot) on the ACT
            # queue, so the first Wh_e projection isn't serialized behind
            # the full weight load.
            qt_sb = constp.tile([128, KC, B, TC], bf16)
            nc.sync.dma_start(qt_sb[:], qt_d[:])
            wst = constp.tile([128, KC, H], bf16)
            nc.sync.dma_start(wst[:], wst_d[:])
            wht = constp.tile([128, KC, H], bf16)
            nc.scalar.dma_start(wht[:], wht_d[:])
            v_sb = constp.tile([128, KC], bf16)
            nc.scalar.dma_start(v_sb[:], v_d[:])
            wot = constp.tile([128, 2 * KC, H], bf16)
            nc.scalar.dma_start(wot[:], wot_d[:])

            # ---- Ws_q^T for all (b, t), duplicated into pairs:
            # qs2[o-part, oc, b, t, j] = Ws_q[o, b, t] for j in {0,1}.
            # The pair duplication keeps the broadcast tensor_tensor's qs
            # operand innermost AP at [1, 2] (2-byte step-1) => 2x_1P mode.
            qs2 = constp.tile([128, OC, B, TC, 2], bf16)
            for oc in range(OC):
                ps = psA.tile([128, B * TC], f32, tag="proj")
                for kc in range(KC):
                    nc.tensor.matmul(
                        ps[:],
                        wst[:, kc, oc * 128 : (oc + 1) * 128],
                        qt_sb[:, kc, :, :],
                        start=(kc == 0),
                        stop=(kc == KC - 1),
                    )
                nc.vector.tensor_copy(
                    qs2[:, oc].rearrange("p b t two -> p (b t) two"),
                    ps[:].unsqueeze(2).broadcast_to([128, B * TC, 2]),
                )

            # Software-pipelined emission: engines execute their streams in
            # order; tail(b-1) is emitted in segments interleaved between
            # head(b)'s per-oc add groups.
            state = {}

            def head_start(b):
                L = Ls[b]
                # load encoder (both layouts), full-S tiles for clean DMA
                encT_b = encp.tile([128, KC, S], bf16, tag="encT")
                nc.sync.dma_start(encT_b[:], encT_d[b])
                enc_b = encp.tile([128, S // 128, H], bf16, tag="enc")
                nc.scalar.dma_start(enc_b[:], enc_d[b])

                # Wh_e^T: es[o-part, s] per oc
                es = []
                for oc in range(OC):
                    ps = psA.tile([128, L], f32, tag="proj")
                    for kc in range(KC):
                        nc.tensor.matmul(
                            ps[:],
                            wht[:, kc, oc * 128 : (oc + 1) * 128],
                            encT_b[:, kc, :L],
                            start=(kc == 0),
                            stop=(kc == KC - 1),
                        )
                    e = esp.tile([128, L], bf16, tag=f"es{oc}")
                    nc.vector.tensor_copy(e[:], ps[:])
                    es.append(e)
                state[b] = ([[None, None] for _ in range(OC)], enc_b, es)

            def head_adds(b, oc):
                """Broadcast-TT adds + tanh for one oc of batch b."""
                L = Ls[b]
                X, enc_b, es = state[b]
                halves = 2 if L >= 128 else 1
                HT = TC // halves
                G = min(HT, 4 if L >= 192 else 8 if L >= 96 else 16)
                for h2 in range(halves):
                    x = xp.tile([128, HT, L], bf16, tag=f"x{oc}h{h2}",
                                name=f"x{oc}h{h2}")
                    for c0 in range(0, HT, G):
                        t0 = h2 * HT + c0
                        in0 = (
                            es[oc][:]
                            .unsqueeze(1)
                            .broadcast_to([128, G, L])
                            .rearrange("p g (h two) -> p g h two", two=2)
                        )
                        in1 = (
                            qs2[:, oc, b, t0 : t0 + G, :]
                            .unsqueeze(2)
                            .broadcast_to([128, G, L // 2, 2])
                        )
                        out = x[:, c0 : c0 + G, :].rearrange(
                            "p g (h two) -> p g h two", two=2
                        )
                        nc.vector.tensor_tensor(out, in0, in1, mybir.AluOpType.add)
                    if stages != 4:
                        nc.scalar.activation(x[:], x[:], AF.Tanh)
                    X[oc][h2] = x
                if halves == 1:
                    X[oc][1] = X[oc][0]

            def xslice(b, t):
                """X[o, t, :] SBUF slice for target t of batch b, per oc."""
                X, _, _ = state[b]
                L = Ls[b]
                halves = 2 if L >= 128 else 1
                HT = TC // halves
                return [X[oc][t // HT][:, t % HT, :] for oc in range(OC)]

            def tail_vdot(b):
                L = Ls[b]
                if stages == 1:
                    return
                # energies[t, s] = sum_o v_o X[o, t, s]: M=1 matmuls, 8 t's
                # per PSUM tile (4 col groups x 2 bank slots), wide evac,
                # partition->partition DMA gather. Accumulation groups
                # sharing a (partition, bank) zero-region must not
                # interleave; col groups (distinct partitions) may.
                energ = smp.tile([32, L], f32, tag="energ")
                for h in range(TC // 8):
                    psq = psV.tile([128, 2, 512 // 2], f32, tag="vdot")
                    for n in range(2):
                        for oc in range(OC):
                            for j in range(4):
                                t = h * 8 + 4 * n + j
                                xs = xslice(b, t)
                                nc.tensor.matmul(
                                    psq[32 * j : 32 * j + 1, n, :L],
                                    v_sb[:, oc : oc + 1],
                                    xs[oc],
                                    start=(oc == 0),
                                    stop=(oc == OC - 1),
                                    tile_position=(0, 32 * j),
                                )
                    vscr = smp.tile([128, 2, L], f32, tag="vscr")
                    nc.vector.tensor_copy(vscr[:], psq[:, :, :L])
                    vsr = vscr.rearrange("(g r) n f -> g r n f", r=32)
                    for n in range(2):
                        nc.sync.dma_start(
                            energ[h * 8 + 4 * n : h * 8 + 4 * n + 4, :],
                            vsr[:, 0, n, :],
                        )
                state[b] = state[b] + (energ,)

            def tail_softmax(b):
                L = Ls[b]
                ln = min(int(lens[b]), S)
                SC = (L + 127) // 128
                L128 = SC * 128
                if stages < 2:
                    return
                energ = state[b][3]
                if stages == 2:
                    ob = outp.tile([32, 16], f32, tag="ob1")
                    nc.vector.tensor_copy(ob[:], energ[:, :16])
                    nc.sync.dma_start(out_d[b][:, :16], ob[:])
                    return
                # softmax over s (energies bounded by sum|v| ~ 20: raw exp
                # is safe — skip max-subtract). The s >= len_b mask is
                # applied by exp-ing only the first ln columns and zeroing
                # the weight tail.
                w_sb = smp.tile([32, L128], bf16, tag="w")
                if L128 > ln:
                    nc.vector.memset(w_sb[:, ln:], 0.0)
                sm = smp.tile([32, 1], f32, tag="sm")
                nc.scalar.activation(
                    w_sb[:, :ln], energ[:, :ln], AF.Exp, accum_out=sm[:]
                )
                rs = smp.tile([32, 1], f32, tag="rs")
                nc.vector.reciprocal(rs[:], sm[:])
                nc.vector.tensor_scalar_mul(w_sb[:, :ln], w_sb[:, :ln], rs[:])

                # w^T via DMA xbar transpose: [32, L128] -> [L128, 32]
                wT = smp.tile([128, SC, TC], bf16, tag="wT")
                for sc in range(SC):
                    nc.sync.dma_start_transpose(
                        wT[:, sc, :], w_sb[:, sc * 128 : (sc + 1) * 128]
                    )
                state[b] = state[b][:4] + (wT,)

            def tail_out(b):
                L = Ls[b]
                SC = (L + 127) // 128
                if stages < 3:
                    return
                enc_b = state[b][1]
                wT = state[b][4]
                # ctx^T[h, t] = sum_s enc[s, h] * w[t, s]; all 4 oc slots in
                # one PSUM bank-row (groups are sequential per slot — legal)
                psc = psC.tile([128, OC, TC], f32, tag="ctx")
                for oc in range(OC):
                    for sc in range(SC):
                        nc.tensor.matmul(
                            psc[:, oc, :],
                            enc_b[:, sc, oc * 128 : (oc + 1) * 128],
                            wT[:, sc, :],
                            start=(sc == 0),
                            stop=(sc == SC - 1),
                        )
                ctxT = outp.tile([128, OC, TC], bf16, tag="ctxT")
                nc.vector.tensor_copy(ctxT[:], psc[:])

                # out[t, o] = tanh(sum_k comb^T[k, t] * W_out[o, k])
                pso = psO.tile([32, H], f32, tag="outp")
                for kc in range(2 * KC):
                    lhsT = (
                        ctxT[:, kc, :]
                        if kc < OC
                        else qt_sb[:, kc - OC, b, :]
                    )
                    nc.tensor.matmul(
                        pso[:],
                        lhsT,
                        wot[:, kc, :],
                        start=(kc == 0),
                        stop=(kc == 2 * KC - 1),
                    )
                ob = outp.tile([32, H], f32, tag="ob")
                nc.scalar.activation(ob[:], pso[:], AF.Tanh)
                nc.sync.dma_start(out_d[b], ob[:])
                del state[b]

            # Descending-L order: the pipeline tail drain (last batch's
            # tail with no head to overlap) is paid on the smallest batch.
            order = sorted(range(B), key=lambda b: -Ls[b])
            prev = None
            for b in order:
                head_start(b)
                head_adds(b, 0)
                head_adds(b, 1)
                if prev is not None:
                    tail_vdot(prev)
                head_adds(b, 2)
                if prev is not None:
                    tail_softmax(prev)
                head_adds(b, 3)
                if prev is not None:
                    tail_out(prev)
                prev = b
            tail_vdot(prev)
            tail_softmax(prev)
            tail_out(prev)

    nc.compile()
    return nc


def _prep_inputs(query, encoder_outputs, src_lengths, W_s, W_h, v, W_out):
    """Host-side: cast to bf16 and pre-arrange into SBUF layouts."""
    q = np.asarray(query, np.float32)
    e = np.asarray(encoder_outputs, np.float32)

    # [128, KC, B, TC] per core: qt[p, kc, b, t] = q[b, c*TC+t, kc*128+p]
    # build once for full T then slice per core.
    qt_full = np.transpose(
        q.reshape(B, T, KC, 128), (2, 3, 0, 1)
    )  # [KC, 128, B, T]
    qt_full = np.ascontiguousarray(np.swapaxes(qt_full, 0, 1)).astype(_BF16)
    # -> [128, KC, B, T]

    # encT[b, p, kc, s] = e[b, s, kc*128+p]
    encT = np.ascontiguousarray(
        np.transpose(e.reshape(B, S, KC, 128), (0, 3, 2, 1))
    ).astype(_BF16)
    # enc[b, p, sc, h] = e[b, sc*128+p, h]
    enc = np.ascontiguousarray(
        np.transpose(e.reshape(B, S // 128, 128, H), (0, 2, 1, 3))
    ).astype(_BF16)

    # wst[p, kc, o] = W_s[o, kc*128+p]
    wst = np.ascontiguousarray(
        np.transpose(np.asarray(W_s, np.float32).reshape(H, KC, 128), (2, 1, 0))
    ).astype(_BF16)
    wht = np.ascontiguousarray(
        np.transpose(np.asarray(W_h, np.float32).reshape(H, KC, 128), (2, 1, 0))
    ).astype(_BF16)
    # v[p, kc] = v[kc*128+p]
    v_pre = np.ascontiguousarray(
        np.asarray(v, np.float32).reshape(KC, 128).T
    ).astype(_BF16)
    # wot[p, kc, o] = W_out[o, kc*128+p]   (k = 2H contraction)
    wot = np.ascontiguousarray(
        np.transpose(np.asarray(W_out, np.float32).reshape(H, 2 * KC, 128), (2, 1, 0))
    ).astype(_BF16)

    lens = tuple(int(x) for x in np.asarray(src_lengths).reshape(-1))
    return qt_full, encT, enc, wst, wht, v_pre, wot, lens


def kernel(query, encoder_outputs, src_lengths, W_s, W_h, v, W_out):
    from concourse import bass_utils

    qt_full, encT, enc, wst, wht, v_pre, wot, lens = _prep_inputs(
        query, encoder_outputs, src_lengths, W_s, W_h, v, W_out
    )
    nc = _build(lens)

    in_maps = []
    for c in range(NCORES):
        qt_c = np.ascontiguousarray(
            qt_full[:, :, :, c * TC : (c + 1) * TC]
        )
        in_maps.append(
            {
                "qt": qt_c,
                "encT": encT,
                "enc": enc,
                "wst": wst,
                "wht": wht,
                "v": v_pre,
                "wot": wot,
            }
        )

    res = bass_utils.run_bass_kernel_spmd(nc, in_maps, core_ids=list(range(NCORES)))

    out = np.empty((B, T, H), np.float32)
    for c in range(NCORES):
        out[:, c * TC : (c + 1) * TC, :] = res.results[c]["out"]
    return out


# revision 28
# speedup vs baseline: 1.2659x; 1.0519x over previous
"""Bahdanau attention Trainium2 kernel.

Problem: B=8, T=256, S=256, H=512 (fp32 I/O).
  Ws_q = q @ W_s.T ; Wh_e = e @ W_h.T
  energies[b,t,s] = v . tanh(Ws_q[b,t,:] + Wh_e[b,s,:])   (masked s >= len_b)
  attn = softmax_s(energies); ctx = attn @ e
  out = tanh(concat([ctx, q]) @ W_out.T)

Sharding: sequence-parallel over T — core c handles t in [c*32, (c+1)*32)
for ALL batches, balancing src_lengths sparsity across cores.

Per-core dataflow (bf16 compute, fp32 accumulation):
  PE   : Ws_q^T [o,t] and Wh_e^T [o,s] projections (o on partitions)
  DVE  : X[o, t, s] = es[o,s] + qs[o,t] via BROADCAST tensor_tensor:
         es with stride-0 over t, qs pre-duplicated into pairs
         (qs2[o,t,2]) so the qs operand's innermost AP dim is [1,2] —
         this keeps every operand 2-byte/step-1 and the DVE in 2x_1P
         mode (~0.52 ns/free-elem vs 0.83 for per-t tensor_scalar).
         Chunked G t's per instruction (F = G*L in [1024, 2048]).
  ACT  : tanh(X) in place, F = 16L per instr (bf16 activation runs at
         ~2 elem/cycle; F~4096 is the sweet spot at ~0.53 ns/elem)
  PE   : energies[t,s] = sum_o v_o X[o,t,s] — M=1 matmuls col-tiled 4-wide
  DMA  : gather PSUM rows {0,32,64,96} -> energies [32t, s]
  DVE/ACT: masked softmax (exp over the first len_b cols + zeroed weight
         tail, with accum_out for the row sums)
  DMA  : xbar-transpose of weights [32,s] -> [s,32]
  PE   : ctx^T[h,t] = enc^T @ w^T ; out[t,o] = tanh(comb^T.T @ W_out^T)

Emission interleaves tail(b-1) segments between head(b)'s per-oc adds so
each engine's in-order queue reaches tail work early (vdot/vscr after
oc1, softmax after oc2, ctx/out after oc3) instead of queueing the whole
tail behind 30us of adds.

HW notes: PSUM accumulation groups must not interleave within a
(partition, bank) zero-region; DMA cannot read PSUM; single-DMA
free-dim->partition scatter silently misplaces data; energ-style
[4, 256] partition-gather DMA ~500ns, [32,128] xbar transpose ~1.3us.
"""

import functools

import ml_dtypes
import numpy as np

B, T, S, H = 8, 256, 256, 512
NCORES = 8
TC = T // NCORES  # 32 target positions per core
KC = H // 128     # 4 contraction chunks
OC = H // 128     # 4 output-feature chunks

_BF16 = ml_dtypes.bfloat16


def _ceil4(x: int) -> int:
    return max(4, (x + 3) // 4 * 4)


@functools.lru_cache(maxsize=8)
def _build(lens: tuple, loop_n: int | None = None, stages: int = 3):
    """Build + compile the per-core Bass program with per-batch s-extents
    baked in. Same program runs on all 8 cores (inputs differ)."""
    import concourse.mybir as mybir
    import concourse.tile as tile
    from concourse import bacc

    f32 = mybir.dt.float32
    bf16 = mybir.dt.bfloat16
    AF = mybir.ActivationFunctionType

    Ls = [_ceil4(l) for l in lens]

    nc = bacc.Bacc("TRN2", target_bir_lowering=False, debug=False)

    # All inputs are host-pre-arranged into SBUF layout [128, free].
    qt_d = nc.dram_tensor("qt", [128, KC, B, TC], bf16, kind="ExternalInput")
    encT_d = nc.dram_tensor("encT", [B, 128, KC, S], bf16, kind="ExternalInput")
    enc_d = nc.dram_tensor("enc", [B, 128, S // 128, H], bf16, kind="ExternalInput")
    wst_d = nc.dram_tensor("wst", [128, KC, H], bf16, kind="ExternalInput")
    wht_d = nc.dram_tensor("wht", [128, KC, H], bf16, kind="ExternalInput")
    v_d = nc.dram_tensor("v", [128, KC], bf16, kind="ExternalInput")
    wot_d = nc.dram_tensor("wot", [128, 2 * KC, H], bf16, kind="ExternalInput")
    out_d = nc.dram_tensor("out", [B, TC, H], f32, kind="ExternalOutput")

    import contextlib

    with tile.TileContext(nc) as tc:
        loop_cm = (
            tc.For_i(
                0, loop_n, 1,
                hint_engines=(
                    mybir.EngineType.PE, mybir.EngineType.DVE,
                    mybir.EngineType.Activation, mybir.EngineType.SP,
                    mybir.EngineType.Pool,
                ),
            )
            if loop_n is not None
            else contextlib.nullcontext()
        )
        with (
            tc.tile_pool(name="const", bufs=1) as constp,
            tc.tile_pool(name="enc", bufs=3) as encp,
            tc.tile_pool(name="es", bufs=2) as esp,
            tc.tile_pool(name="x", bufs=2) as xp,
            tc.tile_pool(name="sm", bufs=3) as smp,
            tc.tile_pool(name="outs", bufs=3) as outp,
            tc.tile_pool(name="psA", bufs=3, space="PSUM") as psA,
            tc.tile_pool(name="psV", bufs=2, space="PSUM") as psV,
            tc.tile_pool(name="psC", bufs=1, space="PSUM") as psC,
            tc.tile_pool(name="psO", bufs=1, space="PSUM") as psO,
            loop_cm,
        ):
            # ---- persistent weights/activations ----
            # Two HWDGE queues in parallel: projQ deps (qt, wst) on the SP
            # queue; projE dep (wht) + late consts (v, wot) on the ACT
            # queue, so the first Wh_e projection isn't serialized behind
            # the full weight load.
            # Fill-critical DMAs split across both HWDGE queues: wht + batch
            # 0's encT on the ACT queue (idle during fill; LoadActFuncSet
            # queues behind them, still long before the first tanh), qt +
            # wst on the SP queue. Steady-state DMAs all go on SP so issue
            # costs never eat into the tanh-bound ACT budget.
            wht = constp.tile([128, KC, H], bf16)
            nc.scalar.dma_start(wht[:], wht_d[:])
            qt_sb = constp.tile([128, KC, B, TC], bf16)
            nc.sync.dma_start(qt_sb[:], qt_d[:])
            wst = constp.tile([128, KC, H], bf16)
            nc.sync.dma_start(wst[:], wst_d[:])
            # v/wot are not needed until the first tail; their DMA issue is
            # deferred below so batch 0's encT/enc aren't queued behind them.
            v_sb = constp.tile([128, KC], bf16)
            wot = constp.tile([128, 2 * KC, H], bf16)

            # ---- Ws_q^T for all (b, t), duplicated into pairs:
            # qs2[o-part, oc, b, t, j] = Ws_q[o, b, t] for j in {0,1}.
            # The pair duplication keeps the broadcast tensor_tensor's qs
            # operand innermost AP at [1, 2] (2-byte step-1) => 2x_1P mode.
            # Per-oc tiles: a single qs2 tile would make the first adds
            # falsely wait on all 4 evacs (tile-granular dep tracking).
            qs2 = []

            def emit_qsproj_oc(oc):
                q2 = constp.tile([128, B, TC, 2], bf16, tag=f"qs2_{oc}",
                                 name=f"qs2_{oc}")
                ps = psA.tile([128, B * TC], f32, tag="proj")
                for kc in range(KC):
                    nc.tensor.matmul(
                        ps[:],
                        wst[:, kc, oc * 128 : (oc + 1) * 128],
                        qt_sb[:, kc, :, :],
                        start=(kc == 0),
                        stop=(kc == KC - 1),
                    )
                nc.vector.tensor_copy(
                    q2[:].rearrange("p b t two -> p (b t) two"),
                    ps[:].unsqueeze(2).broadcast_to([128, B * TC, 2]),
                )
                qs2.append(q2)

            # Software-pipelined emission: engines execute their streams in
            # order; tail(b-1) is emitted in segments interleaved between
            # head(b)'s per-oc add groups.
            state = {}
            aux = {}

            def head_dma(b, first=False):
                # load encoder (both layouts), full-S tiles for clean DMA
                encT_b = encp.tile([128, KC, S], bf16, tag="encT")
                (nc.scalar if first else nc.sync).dma_start(encT_b[:], encT_d[b])
                enc_b = encp.tile([128, S // 128, H], bf16, tag="enc")
                nc.sync.dma_start(enc_b[:], enc_d[b])
                state[b] = ([None] * OC, enc_b, [None] * OC, encT_b)

            def head_esproj(b, oc):
                # Wh_e^T: es[o-part, s] for one oc
                L = Ls[b]
                _, _, es, encT_b = state[b]
                ps = psA.tile([128, L], f32, tag="proj")
                for kc in range(KC):
                    nc.tensor.matmul(
                        ps[:],
                        wht[:, kc, oc * 128 : (oc + 1) * 128],
                        encT_b[:, kc, :L],
                        start=(kc == 0),
                        stop=(kc == KC - 1),
                    )
                e = esp.tile([128, L], bf16, tag=f"es{oc}")
                nc.vector.tensor_copy(e[:], ps[:])
                es[oc] = e

            def head_adds(b, oc):
                """Broadcast-TT adds + one tanh for one oc of batch b."""
                L = Ls[b]
                X, enc_b, es, _ = state[b]
                G = min(TC, 16 if L >= 128 else 32)
                x = xp.tile([128, TC, L], bf16, tag=f"x{oc}", name=f"x{oc}")
                for t0 in range(0, TC, G):
                    in0 = (
                        es[oc][:]
                        .unsqueeze(1)
                        .broadcast_to([128, G, L])
                        .rearrange("p g (h two) -> p g h two", two=2)
                    )
                    in1 = (
                        qs2[oc][:, b, t0 : t0 + G, :]
                        .unsqueeze(2)
                        .broadcast_to([128, G, L // 2, 2])
                    )
                    out = x[:, t0 : t0 + G, :].rearrange(
                        "p g (h two) -> p g h two", two=2
                    )
                    nc.vector.tensor_tensor(out, in0, in1, mybir.AluOpType.add)
                if stages != 4:
                    nc.scalar.activation(x[:], x[:], AF.Tanh)
                X[oc] = x

            def xslice(b, t):
                """X[o, t, :] SBUF slice for target t of batch b, per oc."""
                X, _, _, _ = state[b]
                return [X[oc][:, t, :] for oc in range(OC)]

            def tail_vdot(b, t0=0, nt=TC, part=""):
                L = Ls[b]
                if stages == 1:
                    return
                # energies[t, s] = sum_o v_o X[o, t, s]: M=1 matmuls, 8 t's
                # per PSUM tile (4 col groups x 2 bank slots), wide evac,
                # partition->partition DMA gather. Accumulation groups
                # sharing a (partition, bank) zero-region must not
                # interleave; col groups (distinct partitions) may.
                energ = smp.tile([nt, L], f32, tag=f"energ{part}")
                for h in range(nt // 8):
                    psq = psV.tile([128, 2, 512 // 2], f32, tag="vdot")
                    for n in range(2):
                        for oc in range(OC):
                            for j in range(4):
                                t = t0 + h * 8 + 4 * n + j
                                xs = xslice(b, t)
                                nc.tensor.matmul(
                                    psq[32 * j : 32 * j + 1, n, :L],
                                    v_sb[:, oc : oc + 1],
                                    xs[oc],
                                    start=(oc == 0),
                                    stop=(oc == OC - 1),
                                    tile_position=(0, 32 * j),
                                )
                    vscr = smp.tile([128, 2, L], f32, tag="vscr")
                    nc.vector.tensor_copy(vscr[:], psq[:, :, :L])
                    vsr = vscr.rearrange("(g r) n f -> g r n f", r=32)
                    for n in range(2):
                        nc.sync.dma_start(
                            energ[h * 8 + 4 * n : h * 8 + 4 * n + 4, :],
                            vsr[:, 0, n, :],
                        )
                aux[(b, part)] = {"energ": energ}

            def tail_softmax(b, t0=0, nt=TC, part=""):
                L = Ls[b]
                ln = min(int(lens[b]), S)
                SC = (L + 127) // 128
                L128 = SC * 128
                if stages < 2:
                    return
                energ = aux[(b, part)]["energ"]
                if stages == 2:
                    ob = outp.tile([32, 16], f32, tag="ob1")
                    nc.vector.tensor_copy(ob[:], energ[:, :16])
                    nc.sync.dma_start(out_d[b][:, :16], ob[:])
                    return
                # softmax over s (energies bounded by sum|v| ~ 20: raw exp
                # is safe — skip max-subtract). The s >= len_b mask is
                # applied by exp-ing only the first ln columns and zeroing
                # the weight tail.
                w_sb = smp.tile([nt, L128], bf16, tag=f"w{part}")
                if L128 > ln:
                    nc.vector.memset(w_sb[:, ln:], 0.0)
                sm = smp.tile([nt, 1], f32, tag=f"sm{part}")
                nc.scalar.activation(
                    w_sb[:, :ln], energ[:, :ln], AF.Exp, accum_out=sm[:]
                )
                rs = smp.tile([nt, 1], f32, tag=f"rs{part}")
                nc.vector.reciprocal(rs[:], sm[:])
                nc.vector.tensor_scalar_mul(w_sb[:, :ln], w_sb[:, :ln], rs[:])

                # w^T via DMA xbar transpose: [nt, L128] -> [L128, nt]
                wT = smp.tile([128, SC, nt], bf16, tag=f"wT{part}")
                for sc in range(SC):
                    nc.sync.dma_start_transpose(
                        wT[:, sc, :], w_sb[:, sc * 128 : (sc + 1) * 128]
                    )
                aux[(b, part)]["wT"] = wT

            def tail_out(b, t0=0, nt=TC, part=""):
                L = Ls[b]
                SC = (L + 127) // 128
                if stages < 3:
                    return
                enc_b = state[b][1]
                wT = aux[(b, part)]["wT"]
                # ctx^T[h, t] = sum_s enc[s, h] * w[t, s]; all 4 oc slots in
                # one PSUM bank-row (groups are sequential per slot — legal)
                psc_full = psC.tile([128, OC, TC], f32, tag="ctx")
                psc = psc_full[:, :, :nt]
                for oc in range(OC):
                    for sc in range(SC):
                        nc.tensor.matmul(
                            psc[:, oc, :],
                            enc_b[:, sc, oc * 128 : (oc + 1) * 128],
                            wT[:, sc, :],
                            start=(sc == 0),
                            stop=(sc == SC - 1),
                        )
                ctxT = outp.tile([128, OC, nt], bf16, tag=f"ctxT{part}")
                nc.vector.tensor_copy(ctxT[:], psc[:])

                # out[t, o] = tanh(sum_k comb^T[k, t] * W_out[o, k])
                pso_full = psO.tile([32, H], f32, tag="outp")
                pso = pso_full[:nt, :]
                for kc in range(2 * KC):
                    lhsT = (
                        ctxT[:, kc, :]
                        if kc < OC
                        else qt_sb[:, kc - OC, b, t0 : t0 + nt]
                    )
                    nc.tensor.matmul(
                        pso[:],
                        lhsT,
                        wot[:, kc, :],
                        start=(kc == 0),
                        stop=(kc == 2 * KC - 1),
                    )
                ob = outp.tile([nt, H], f32, tag=f"ob{part}")
                nc.scalar.activation(ob[:], pso[:], AF.Tanh)
                nc.sync.dma_start(out_d[b][t0 : t0 + nt], ob[:])
                del aux[(b, part)]
                if t0 + nt == TC:
                    del state[b]

            # Descending-L order: the pipeline tail drain (last batch's
            # tail with no head to overlap) is paid on the smallest batch.
            order = sorted(range(B), key=lambda b: -Ls[b])
            # Batch 0 fill: per-oc round-robin of es-proj / qs-proj / adds
            # so the first tanh starts as soon as es[0]+qs2[0] exist,
            # instead of after all 8 projections and evacs.
            b0 = order[0]
            head_dma(b0, first=True)
            for oc in range(OC):
                head_esproj(b0, oc)
                emit_qsproj_oc(oc)
                if oc == OC - 1:
                    nc.sync.dma_start(v_sb[:], v_d[:])
                    nc.sync.dma_start(wot[:], wot_d[:])
                head_adds(b0, oc)
            prev = b0
            for b in order[1:]:
                head_dma(b)
                for oc in range(OC):
                    head_esproj(b, oc)
                head_adds(b, 0)
                head_adds(b, 1)
                tail_vdot(prev)
                head_adds(b, 2)
                tail_softmax(prev)
                head_adds(b, 3)
                tail_out(prev)
                prev = b
            # Final batch: two t-halves pipelined so half a's softmax/
            # ctx/out overlaps half b's vdot instead of a serial drain.
            HT = TC // 2
            tail_vdot(prev, 0, HT, "a")
            tail_softmax(prev, 0, HT, "a")
            tail_vdot(prev, HT, HT, "b")
            tail_out(prev, 0, HT, "a")
            tail_softmax(prev, HT, HT, "b")
            tail_out(prev, HT, HT, "b")

    nc.compile()
    return nc


def _prep_inputs(query, encoder_outputs, src_lengths, W_s, W_h, v, W_out):
    """Host-side: cast to bf16 and pre-arrange into SBUF layouts."""
    q = np.asarray(query, np.float32)
    e = np.asarray(encoder_outputs, np.float32)

    # [128, KC, B, TC] per core: qt[p, kc, b, t] = q[b, c*TC+t, kc*128+p]
    # build once for full T then slice per core.
    qt_full = np.transpose(
        q.reshape(B, T, KC, 128), (2, 3, 0, 1)
    )  # [KC, 128, B, T]
    qt_full = np.ascontiguousarray(np.swapaxes(qt_full, 0, 1)).astype(_BF16)
    # -> [128, KC, B, T]

    # encT[b, p, kc, s] = e[b, s, kc*128+p]
    encT = np.ascontiguousarray(
        np.transpose(e.reshape(B, S, KC, 128), (0, 3, 2, 1))
    ).astype(_BF16)
    # enc[b, p, sc, h] = e[b, sc*128+p, h]
    enc = np.ascontiguousarray(
        np.transpose(e.reshape(B, S // 128, 128, H), (0, 2, 1, 3))
    ).astype(_BF16)

    # wst[p, kc, o] = W_s[o, kc*128+p]
    wst = np.ascontiguousarray(
        np.transpose(np.asarray(W_s, np.float32).reshape(H, KC, 128), (2, 1, 0))
    ).astype(_BF16)
    wht = np.ascontiguousarray(
        np.transpose(np.asarray(W_h, np.float32).reshape(H, KC, 128), (2, 1, 0))
    ).astype(_BF16)
    # v[p, kc] = v[kc*128+p]
    v_pre = np.ascontiguousarray(
        np.asarray(v, np.float32).reshape(KC, 128).T
    ).astype(_BF16)
    # wot[p, kc, o] = W_out[o, kc*128+p]   (k = 2H contraction)
    wot = np.ascontiguousarray(
        np.transpose(np.asarray(W_out, np.float32).reshape(H, 2 * KC, 128), (2, 1, 0))
    ).astype(_BF16)

    lens = tuple(int(x) for x in np.asarray(src_lengths).reshape(-1))
    return qt_full, encT, enc, wst, wht, v_pre, wot, lens


def kernel(query, encoder_outputs, src_lengths, W_s, W_h, v, W_out):
    from concourse import bass_utils

    qt_full, encT, enc, wst, wht, v_pre, wot, lens = _prep_inputs(
        query, encoder_outputs, src_lengths, W_s, W_h, v, W_out
    )
    nc = _build(lens)

    in_maps = []
    for c in range(NCORES):
        qt_c = np.ascontiguousarray(
            qt_full[:, :, :, c * TC : (c + 1) * TC]
        )
        in_maps.append(
            {
                "qt": qt_c,
                "encT": encT,
                "enc": enc,
                "wst": wst,
                "wht": wht,
                "v": v_pre,
                "wot": wot,
            }
        )

    res = bass_utils.run_bass_kernel_spmd(nc, in_maps, core_ids=list(range(NCORES)))

    out = np.empty((B, T, H), np.float32)
    for c in range(NCORES):
        out[:, c * TC : (c + 1) * TC, :] = res.results[c]["out"]
    return out


# revision 30
# speedup vs baseline: 1.3038x; 1.0299x over previous
"""Bahdanau attention Trainium2 kernel.

Problem: B=8, T=256, S=256, H=512 (fp32 I/O).
  Ws_q = q @ W_s.T ; Wh_e = e @ W_h.T
  energies[b,t,s] = v . tanh(Ws_q[b,t,:] + Wh_e[b,s,:])   (masked s >= len_b)
  attn = softmax_s(energies); ctx = attn @ e
  out = tanh(concat([ctx, q]) @ W_out.T)

Sharding: sequence-parallel over T — core c handles t in [c*32, (c+1)*32)
for ALL batches, balancing src_lengths sparsity across cores.

Per-core dataflow (bf16 compute, fp32 accumulation):
  PE   : Ws_q^T [o,t] and Wh_e^T [o,s] projections (o on partitions)
  DVE  : X[o, t, s] = es[o,s] + qs[o,t] via BROADCAST tensor_tensor:
         es with stride-0 over t, qs pre-duplicated into pairs
         (qs2[o,t,2]) so the qs operand's innermost AP dim is [1,2] —
         this keeps every operand 2-byte/step-1 and the DVE in 2x_1P
         mode (~0.52 ns/free-elem vs 0.83 for per-t tensor_scalar).
         Chunked G t's per instruction (F = G*L in [1024, 2048]).
  ACT  : tanh(X) in place, F = 16L per instr (bf16 activation runs at
         ~2 elem/cycle; F~4096 is the sweet spot at ~0.53 ns/elem)
  PE   : energies[t,s] = sum_o v_o X[o,t,s] — M=1 matmuls col-tiled 4-wide
  DMA  : gather PSUM rows {0,32,64,96} -> energies [32t, s]
  DVE/ACT: masked softmax (exp over the first len_b cols + zeroed weight
         tail, with accum_out for the row sums)
  DMA  : xbar-transpose of weights [32,s] -> [s,32]
  PE   : ctx^T[h,t] = enc^T @ w^T ; out[t,o] = tanh(comb^T.T @ W_out^T)

Emission interleaves tail(b-1) segments between head(b)'s per-oc adds so
each engine's in-order queue reaches tail work early (vdot/vscr after
oc1, softmax after oc2, ctx/out after oc3) instead of queueing the whole
tail behind 30us of adds.

HW notes: PSUM accumulation groups must not interleave within a
(partition, bank) zero-region; DMA cannot read PSUM; single-DMA
free-dim->partition scatter silently misplaces data; energ-style
[4, 256] partition-gather DMA ~500ns, [32,128] xbar transpose ~1.3us.
"""

import functools

import ml_dtypes
import numpy as np

B, T, S, H = 8, 256, 256, 512
NCORES = 8
TC = T // NCORES  # 32 target positions per core
KC = H // 128     # 4 contraction chunks
OC = H // 128     # 4 output-feature chunks

_BF16 = ml_dtypes.bfloat16


def _ceil4(x: int) -> int:
    return max(4, (x + 3) // 4 * 4)


@functools.lru_cache(maxsize=8)
def _build(lens: tuple, loop_n: int | None = None, stages: int = 3):
    """Build + compile the per-core Bass program with per-batch s-extents
    baked in. Same program runs on all 8 cores (inputs differ)."""
    import concourse.mybir as mybir
    import concourse.tile as tile
    from concourse import bacc

    f32 = mybir.dt.float32
    bf16 = mybir.dt.bfloat16
    AF = mybir.ActivationFunctionType

    Ls = [_ceil4(l) for l in lens]

    nc = bacc.Bacc("TRN2", target_bir_lowering=False, debug=False)

    # All inputs are host-pre-arranged into SBUF layout [128, free].
    qt_d = nc.dram_tensor("qt", [128, KC, B, TC], bf16, kind="ExternalInput")
    encT_d = nc.dram_tensor("encT", [B, 128, KC, S], bf16, kind="ExternalInput")
    enc_d = nc.dram_tensor("enc", [B, 128, S // 128, H], bf16, kind="ExternalInput")
    wst_d = nc.dram_tensor("wst", [128, KC, H], bf16, kind="ExternalInput")
    wht_d = nc.dram_tensor("wht", [128, KC, H], bf16, kind="ExternalInput")
    v_d = nc.dram_tensor("v", [128, KC], bf16, kind="ExternalInput")
    wot_d = nc.dram_tensor("wot", [128, 2 * KC, H], bf16, kind="ExternalInput")
    out_d = nc.dram_tensor("out", [B, TC, H], f32, kind="ExternalOutput")

    import contextlib

    with tile.TileContext(nc) as tc:
        loop_cm = (
            tc.For_i(
                0, loop_n, 1,
                hint_engines=(
                    mybir.EngineType.PE, mybir.EngineType.DVE,
                    mybir.EngineType.Activation, mybir.EngineType.SP,
                    mybir.EngineType.Pool,
                ),
            )
            if loop_n is not None
            else contextlib.nullcontext()
        )
        with (
            tc.tile_pool(name="const", bufs=1) as constp,
            tc.tile_pool(name="enc", bufs=3) as encp,
            tc.tile_pool(name="es", bufs=2) as esp,
            tc.tile_pool(name="x", bufs=2) as xp,
            tc.tile_pool(name="sm", bufs=3) as smp,
            tc.tile_pool(name="outs", bufs=3) as outp,
            tc.tile_pool(name="psA", bufs=3, space="PSUM") as psA,
            tc.tile_pool(name="psV", bufs=2, space="PSUM") as psV,
            tc.tile_pool(name="psC", bufs=1, space="PSUM") as psC,
            tc.tile_pool(name="psO", bufs=1, space="PSUM") as psO,
            loop_cm,
        ):
            # ---- persistent weights/activations ----
            # Two HWDGE queues in parallel: projQ deps (qt, wst) on the SP
            # queue; projE dep (wht) + late consts (v, wot) on the ACT
            # queue, so the first Wh_e projection isn't serialized behind
            # the full weight load.
            # Fill-critical DMAs split across both HWDGE queues: wht + batch
            # 0's encT on the ACT queue (idle during fill; LoadActFuncSet
            # queues behind them, still long before the first tanh), qt +
            # wst on the SP queue. Steady-state DMAs all go on SP so issue
            # costs never eat into the tanh-bound ACT budget.
            wht = constp.tile([128, KC, H], bf16)
            nc.sync.dma_start(wht[:], wht_d[:])
            qt_sb = constp.tile([128, KC, B, TC], bf16)
            nc.scalar.dma_start(qt_sb[:], qt_d[:])
            wst = constp.tile([128, KC, H], bf16)
            nc.sync.dma_start(wst[:], wst_d[:])
            # v/wot are not needed until the first tail; their DMA issue is
            # deferred below so batch 0's encT/enc aren't queued behind them.
            v_sb = constp.tile([128, KC], bf16)
            wot = constp.tile([128, 2 * KC, H], bf16)

            # ---- Ws_q^T for all (b, t), duplicated into pairs:
            # qs2[o-part, oc, b, t, j] = Ws_q[o, b, t] for j in {0,1}.
            # The pair duplication keeps the broadcast tensor_tensor's qs
            # operand innermost AP at [1, 2] (2-byte step-1) => 2x_1P mode.
            # Per-oc tiles: a single qs2 tile would make the first adds
            # falsely wait on all 4 evacs (tile-granular dep tracking).
            qs2 = []

            def emit_qsproj_oc(oc):
                q2 = constp.tile([128, B, TC, 2], bf16, tag=f"qs2_{oc}",
                                 name=f"qs2_{oc}")
                ps = psA.tile([128, B * TC], f32, tag="proj")
                for kc in range(KC):
                    nc.tensor.matmul(
                        ps[:],
                        wst[:, kc, oc * 128 : (oc + 1) * 128],
                        qt_sb[:, kc, :, :],
                        start=(kc == 0),
                        stop=(kc == KC - 1),
                    )
                nc.vector.tensor_copy(
                    q2[:].rearrange("p b t two -> p (b t) two"),
                    ps[:].unsqueeze(2).broadcast_to([128, B * TC, 2]),
                )
                qs2.append(q2)

            # Software-pipelined emission: engines execute their streams in
            # order; tail(b-1) is emitted in segments interleaved between
            # head(b)'s per-oc add groups.
            state = {}
            aux = {}

            def head_dma(b, first=False):
                # load encoder (both layouts), full-S tiles for clean DMA
                encT_b = encp.tile([128, KC, S], bf16, tag="encT")
                (nc.scalar if first else nc.sync).dma_start(encT_b[:], encT_d[b])
                enc_b = encp.tile([128, S // 128, H], bf16, tag="enc")
                nc.sync.dma_start(enc_b[:], enc_d[b])
                state[b] = ([None] * OC, enc_b, [None] * OC, encT_b)

            def head_esproj(b, oc):
                # Wh_e^T: es[o-part, s] for one oc
                L = Ls[b]
                _, _, es, encT_b = state[b]
                ps = psA.tile([128, L], f32, tag="proj")
                for kc in range(KC):
                    nc.tensor.matmul(
                        ps[:],
                        wht[:, kc, oc * 128 : (oc + 1) * 128],
                        encT_b[:, kc, :L],
                        start=(kc == 0),
                        stop=(kc == KC - 1),
                    )
                e = esp.tile([128, L], bf16, tag=f"es{oc}")
                nc.vector.tensor_copy(e[:], ps[:])
                es[oc] = e

            def head_adds(b, oc):
                """Broadcast-TT adds + one tanh for one oc of batch b."""
                L = Ls[b]
                X, enc_b, es, _ = state[b]
                G = min(TC, 16 if L >= 128 else 32)
                x = xp.tile([128, TC, L], bf16, tag=f"x{oc}", name=f"x{oc}")
                for t0 in range(0, TC, G):
                    in0 = (
                        es[oc][:]
                        .unsqueeze(1)
                        .broadcast_to([128, G, L])
                        .rearrange("p g (h two) -> p g h two", two=2)
                    )
                    in1 = (
                        qs2[oc][:, b, t0 : t0 + G, :]
                        .unsqueeze(2)
                        .broadcast_to([128, G, L // 2, 2])
                    )
                    out = x[:, t0 : t0 + G, :].rearrange(
                        "p g (h two) -> p g h two", two=2
                    )
                    nc.vector.tensor_tensor(out, in0, in1, mybir.AluOpType.add)
                if stages != 4:
                    nc.scalar.activation(x[:], x[:], AF.Tanh)
                X[oc] = x

            def xslice(b, t):
                """X[o, t, :] SBUF slice for target t of batch b, per oc."""
                X, _, _, _ = state[b]
                return [X[oc][:, t, :] for oc in range(OC)]

            def tail_vdot(b, t0=0, nt=TC, part=""):
                L = Ls[b]
                if stages == 1:
                    return
                # energies[t, s] = sum_o v_o X[o, t, s]: M=1 matmuls, 8 t's
                # per PSUM tile (4 col groups x 2 bank slots), wide evac,
                # partition->partition DMA gather. Accumulation groups
                # sharing a (partition, bank) zero-region must not
                # interleave; col groups (distinct partitions) may.
                energ = smp.tile([nt, L], f32, tag=f"energ{part}")
                X = state[b][0]
                # Gt targets per M=1 matmul (n = Gt*L <= 512 f32, one PSUM
                # bank row): halves/quarters the v-dot instruction count.
                Gt = 4 if L <= 128 else 2
                TPT = 4 * Gt  # t's per psq tile (4 col groups)
                for h in range(nt // TPT):
                    psq = psV.tile([128, Gt, 512 // Gt], f32, tag="vdot")
                    for j in range(4):
                        tj = t0 + h * TPT + j * Gt
                        for oc in range(OC):
                            nc.tensor.matmul(
                                psq[32 * j : 32 * j + 1, :, :L],
                                v_sb[:, oc : oc + 1],
                                X[oc][:, tj : tj + Gt, :],
                                start=(oc == 0),
                                stop=(oc == OC - 1),
                                tile_position=(0, 32 * j),
                            )
                    vscr = smp.tile([128, Gt, L], f32, tag="vscr")
                    nc.vector.tensor_copy(vscr[:], psq[:, :, :L])
                    vsr = vscr.rearrange("(g r) n f -> g r n f", r=32)
                    ev = energ.rearrange(
                        "(hh j g) f -> hh j g f", j=4, g=Gt
                    )
                    for n in range(Gt):
                        nc.sync.dma_start(ev[h, :, n, :], vsr[:, 0, n, :])
                aux[(b, part)] = {"energ": energ}

            def tail_softmax(b, t0=0, nt=TC, part=""):
                L = Ls[b]
                ln = min(int(lens[b]), S)
                SC = (L + 127) // 128
                L128 = SC * 128
                if stages < 2:
                    return
                energ = aux[(b, part)]["energ"]
                if stages == 2:
                    ob = outp.tile([32, 16], f32, tag="ob1")
                    nc.vector.tensor_copy(ob[:], energ[:, :16])
                    nc.sync.dma_start(out_d[b][:, :16], ob[:])
                    return
                # softmax over s (energies bounded by sum|v| ~ 20: raw exp
                # is safe — skip max-subtract). The s >= len_b mask is
                # applied by exp-ing only the first ln columns and zeroing
                # the weight tail.
                w_sb = smp.tile([nt, L128], bf16, tag=f"w{part}")
                if L128 > ln:
                    nc.vector.memset(w_sb[:, ln:], 0.0)
                sm = smp.tile([nt, 1], f32, tag=f"sm{part}")
                nc.scalar.activation(
                    w_sb[:, :ln], energ[:, :ln], AF.Exp, accum_out=sm[:]
                )
                rs = smp.tile([nt, 1], f32, tag=f"rs{part}")
                nc.vector.reciprocal(rs[:], sm[:])
                nc.vector.tensor_scalar_mul(w_sb[:, :ln], w_sb[:, :ln], rs[:])

                # w^T via DMA xbar transpose: [nt, L128] -> [L128, nt]
                wT = smp.tile([128, SC, nt], bf16, tag=f"wT{part}")
                for sc in range(SC):
                    nc.sync.dma_start_transpose(
                        wT[:, sc, :], w_sb[:, sc * 128 : (sc + 1) * 128]
                    )
                aux[(b, part)]["wT"] = wT

            def tail_out(b, t0=0, nt=TC, part=""):
                L = Ls[b]
                SC = (L + 127) // 128
                if stages < 3:
                    return
                enc_b = state[b][1]
                wT = aux[(b, part)]["wT"]
                # ctx^T[h, t] = sum_s enc[s, h] * w[t, s]; all 4 oc slots in
                # one PSUM bank-row (groups are sequential per slot — legal)
                psc_full = psC.tile([128, OC, TC], f32, tag="ctx")
                psc = psc_full[:, :, :nt]
                for oc in range(OC):
                    for sc in range(SC):
                        nc.tensor.matmul(
                            psc[:, oc, :],
                            enc_b[:, sc, oc * 128 : (oc + 1) * 128],
                            wT[:, sc, :],
                            start=(sc == 0),
                            stop=(sc == SC - 1),
                        )
                ctxT = outp.tile([128, OC, nt], bf16, tag=f"ctxT{part}")
                nc.vector.tensor_copy(ctxT[:], psc[:])

                # out[t, o] = tanh(sum_k comb^T[k, t] * W_out[o, k])
                pso_full = psO.tile([32, H], f32, tag="outp")
                pso = pso_full[:nt, :]
                for kc in range(2 * KC):
                    lhsT = (
                        ctxT[:, kc, :]
                        if kc < OC
                        else qt_sb[:, kc - OC, b, t0 : t0 + nt]
                    )
                    nc.tensor.matmul(
                        pso[:],
                        lhsT,
                        wot[:, kc, :],
                        start=(kc == 0),
                        stop=(kc == 2 * KC - 1),
                    )
                ob = outp.tile([nt, H], f32, tag=f"ob{part}")
                nc.scalar.activation(ob[:], pso[:], AF.Tanh)
                nc.sync.dma_start(out_d[b][t0 : t0 + nt], ob[:])
                del aux[(b, part)]
                if t0 + nt == TC:
                    del state[b]

            # Descending-L order: the pipeline tail drain (last batch's
            # tail with no head to overlap) is paid on the smallest batch.
            order = sorted(range(B), key=lambda b: -Ls[b])
            # Batch 0 fill: per-oc round-robin of es-proj / qs-proj / adds
            # so the first tanh starts as soon as es[0]+qs2[0] exist,
            # instead of after all 8 projections and evacs.
            b0 = order[0]
            head_dma(b0, first=True)
            for oc in range(OC):
                head_esproj(b0, oc)
                emit_qsproj_oc(oc)
                if oc == OC - 1:
                    nc.sync.dma_start(v_sb[:], v_d[:])
                    nc.sync.dma_start(wot[:], wot_d[:])
                head_adds(b0, oc)
            prev = b0
            for b in order[1:]:
                head_dma(b)
                for oc in range(OC):
                    head_esproj(b, oc)
                head_adds(b, 0)
                head_adds(b, 1)
                tail_vdot(prev)
                head_adds(b, 2)
                tail_softmax(prev)
                head_adds(b, 3)
                tail_out(prev)
                prev = b
            # Final batch: two t-halves pipelined so half a's softmax/
            # ctx/out overlaps half b's vdot instead of a serial drain.
            HT = TC // 2
            tail_vdot(prev, 0, HT, "a")
            tail_softmax(prev, 0, HT, "a")
            tail_vdot(prev, HT, HT, "b")
            tail_out(prev, 0, HT, "a")
            tail_softmax(prev, HT, HT, "b")
            tail_out(prev, HT, HT, "b")

    nc.compile()
    return nc


def _prep_inputs(query, encoder_outputs, src_lengths, W_s, W_h, v, W_out):
    """Host-side: cast to bf16 and pre-arrange into SBUF layouts."""
    q = np.asarray(query, np.float32)
    e = np.asarray(encoder_outputs, np.float32)

    # [128, KC, B, TC] per core: qt[p, kc, b, t] = q[b, c*TC+t, kc*128+p]
    # build once for full T then slice per core.
    qt_full = np.transpose(
        q.reshape(B, T, KC, 128), (2, 3, 0, 1)
    )  # [KC, 128, B, T]
    qt_full = np.ascontiguousarray(np.swapaxes(qt_full, 0, 1)).astype(_BF16)
    # -> [128, KC, B, T]

    # encT[b, p, kc, s] = e[b, s, kc*128+p]
    encT = np.ascontiguousarray(
        np.transpose(e.reshape(B, S, KC, 128), (0, 3, 2, 1))
    ).astype(_BF16)
    # enc[b, p, sc, h] = e[b, sc*128+p, h]
    enc = np.ascontiguousarray(
        np.transpose(e.reshape(B, S // 128, 128, H), (0, 2, 1, 3))
    ).astype(_BF16)

    # wst[p, kc, o] = W_s[o, kc*128+p]
    wst = np.ascontiguousarray(
        np.transpose(np.asarray(W_s, np.float32).reshape(H, KC, 128), (2, 1, 0))
    ).astype(_BF16)
    wht = np.ascontiguousarray(
        np.transpose(np.asarray(W_h, np.float32).reshape(H, KC, 128), (2, 1, 0))
    ).astype(_BF16)
    # v[p, kc] = v[kc*128+p]
    v_pre = np.ascontiguousarray(
        np.asarray(v, np.float32).reshape(KC, 128).T
    ).astype(_BF16)
    # wot[p, kc, o] = W_out[o, kc*128+p]   (k = 2H contraction)
    wot = np.ascontiguousarray(
        np.transpose(np.asarray(W_out, np.float32).reshape(H, 2 * KC, 128), (2, 1, 0))
    ).astype(_BF16)

    lens = tuple(int(x) for x in np.asarray(src_lengths).reshape(-1))
    return qt_full, encT, enc, wst, wht, v_pre, wot, lens


def kernel(query, encoder_outputs, src_lengths, W_s, W_h, v, W_out):
    from concourse import bass_utils

    qt_full, encT, enc, wst, wht, v_pre, wot, lens = _prep_inputs(
        query, encoder_outputs, src_lengths, W_s, W_h, v, W_out
    )
    nc = _build(lens)

    in_maps = []
    for c in range(NCORES):
        qt_c = np.ascontiguousarray(
            qt_full[:, :, :, c * TC : (c + 1) * TC]
        )
        in_maps.append(
            {
                "qt": qt_c,
                "encT": encT,
                "enc": enc,
                "wst": wst,
                "wht": wht,
                "v": v_pre,
                "wot": wot,
            }
        )

    res = bass_utils.run_bass_kernel_spmd(nc, in_maps, core_ids=list(range(NCORES)))

    out = np.empty((B, T, H), np.float32)
    for c in range(NCORES):
        out[:, c * TC : (c + 1) * TC, :] = res.results[c]["out"]
    return out
